# revision 47
# baseline (speedup 1.0000x reference)
"""Multi-head attention (B=2, S=2048, D=1024, H=16) on 8 TRN2 NeuronCores.

Sharding: tensor-parallel over heads x data-parallel over batch.
Core c handles batch b = c//4, head group g = c%4 (4 heads, 256 cols).
W_q/W_k/W_v are split column-wise per group, W_o row-wise; each core
produces a partial [S, D] output, reduced on the host (the W_o
contraction is a pure sum over head groups; b_v/b_o folded in on host).

Device kernel (per core), all matmuls bf16 with fp32 PSUM accumulation:
  - K^T, Q^T projections in transposed layout [dk*2, S] (lhsT = W cols,
    rhs = x^T), V in natural layout [S, dk*4+ones] (lhsT = x^T chunks).
  - scores computed transposed: ST[k,q] = (K^T chunk)^T-matmul vs Q^T.
    The two heads of a pair (rows 0:64 / 64:128 of KT/QT) are emitted
    interleaved so their K=64 matmuls run CONCURRENTLY on disjoint PE
    row-groups (tile_position auto-derives from base_partition) --
    halving the logits phase vs sequential per-head emission.
  - softmax without max-subtraction (logits are O(5) here, exp is safe):
    exp on ACT straight out of PSUM with scale=1/sqrt(dk).
  - ctx^T[dk+1, q] accumulated over k-chunks with an ones-augmented V
    (row dk = softmax denominators), normalized via DVE with a gpsimd
    partition-broadcast of the reciprocals.
  - out partial = ctx^T-chunks @ W_o rows, accumulated over the 2
    128-row chunks of the group's 256 W_o rows.
"""

import numpy as np
import ml_dtypes
from contextlib import ExitStack

import concourse.bass as bass
import concourse.tile as tile
from concourse import bacc, mybir
from concourse.bass_utils import run_bass_kernel_spmd

BF16 = mybir.dt.bfloat16
F32 = mybir.dt.float32

D = 1024            # model dim
H = 16              # heads
DK = 64             # head dim
NCORES = 8
GPB = 4             # head groups per batch (= cores per batch)
HPG = H // GPB      # 4 heads per core
HD = HPG * DK       # 256 cols per group
HAUG = DK + 1       # 65: head block width in augmented-V layout
SP = 512            # q-span / free-dim tile
SCALE = 1.0 / np.sqrt(DK)
# Schraudolph exp on DVE: i16 = rne(logit*S1K + S2K); bitcast int16->bf16
# approximates exp(logit*SCALE) with ~+-3.3% sawtooth error that cancels
# in the softmax normalization (denominator sums the same approx values).
SCH_C = 0.0440
S1K = float(SCALE * np.log2(np.e) * 128.0)
S2K = float((127.0 - SCH_C) * 128.0)


def build(S):
    NQS = S // SP       # q spans
    NSC = S // 128      # sequence chunks (k side)
    NDC = D // 128      # model-dim chunks
    SI = SP // 128      # s-chunks per q-span

    nc = bacc.Bacc("TRN2", target_bir_lowering=False, debug=False)
    HA = HPG * HAUG     # 260: augmented V width
    xT_e = nc.dram_tensor("xT", [S // SP, 128, D // 128, SP], BF16, kind="ExternalInput")
    # m-major weight layout: [128, m, c, 128] so each m-half is one
    # contiguous DMA (m=0 is all the prologue needs)
    wq_e = nc.dram_tensor("wq", [128, 2, D // 128, 128], BF16, kind="ExternalInput")
    wk_e = nc.dram_tensor("wk", [128, 2, D // 128, 128], BF16, kind="ExternalInput")
    wv_e = nc.dram_tensor("wv", [128, D // 128, HA], BF16, kind="ExternalInput")
    wo_e = nc.dram_tensor("wo", [128, 2, D], BF16, kind="ExternalInput")
    bq_e = nc.dram_tensor("bq", [128, 2], F32, kind="ExternalInput")
    bk_e = nc.dram_tensor("bk", [128, 2], F32, kind="ExternalInput")
    out_e = nc.dram_tensor("out", [S, D], F32, kind="ExternalOutput")

    ADD = mybir.AluOpType.add
    MULT = mybir.AluOpType.mult
    EXP = mybir.ActivationFunctionType.Exp

    with tile.TileContext(nc) as tc, ExitStack() as ctx:
        const = ctx.enter_context(tc.tile_pool(name="const", bufs=1))
        qpool = ctx.enter_context(tc.tile_pool(name="qpool", bufs=2))
        cpool = ctx.enter_context(tc.tile_pool(name="cpool", bufs=2))
        ptp = ctx.enter_context(tc.tile_pool(name="ptp", bufs=8))
        obp = ctx.enter_context(tc.tile_pool(name="obp", bufs=4))
        smp = ctx.enter_context(tc.tile_pool(name="smp", bufs=4))
        psum = ctx.enter_context(tc.tile_pool(name="psum", bufs=2, space="PSUM"))

        wq_sb = const.tile([128, 2, NDC, 128], BF16, name="wq_sb")
        wk_sb = const.tile([128, 2, NDC, 128], BF16, name="wk_sb")
        wv_sb = const.tile([128, NDC, HA], BF16, name="wv_sb")
        wo_sb = const.tile([128, 2, D], BF16, name="wo_sb")
        bq_sb = const.tile([128, 2], F32, name="bq_sb")
        bk_sb = const.tile([128, 2], F32, name="bk_sb")
        xT_sb = [const.tile([128, NDC, SP], BF16, name=f"xT{q}") for q in range(NQS)]
        KT_sb = const.tile([128, 2, S], BF16, name="KT_sb")
        V_sb = const.tile([128, NSC, HPG * HAUG], BF16, name="V_sb")

        # PE warm-up: ~20 dummy matmuls on never-written SBUF scratch with
        # no DMA dependency -- they start at t~0 and keep the HAM activity
        # window non-idle through the ~11us DMA prologue, so the first real
        # kproj matmuls run at the full 2.4 GHz instead of the cold 1.2.
        warm = const.tile([128, SP], BF16, name="warm")
        nc.vector.memset(warm[:], 0.0)
        wps = psum.tile([128, SP], F32, tag="mm", name="wps")
        for w in range(8):
            nc.tensor.matmul(wps[:], warm[:, 0:128], warm[:],
                             start=(w == 0), stop=(w == 7))

        # input DMAs, in need-order. The first few land on distinct DMA
        # queues and run in parallel; fine granularity on the span-0
        # pieces lets the first kproj matmuls start as early as possible.
        nc.sync.dma_start(wk_sb[:, 0, 0:4, :], wk_e.ap()[:, 0, 0:4, :])
        nc.sync.dma_start(xT_sb[0][:, 0:2, :], xT_e.ap()[0, :, 0:2, :])
        nc.sync.dma_start(bk_sb[:], bk_e.ap())
        nc.sync.dma_start(wk_sb[:, 0, 4:, :], wk_e.ap()[:, 0, 4:, :])
        nc.sync.dma_start(xT_sb[0][:, 2:4, :], xT_e.ap()[0, :, 2:4, :])
        nc.sync.dma_start(xT_sb[0][:, 4:6, :], xT_e.ap()[0, :, 4:6, :])
        nc.sync.dma_start(xT_sb[0][:, 6:, :], xT_e.ap()[0, :, 6:, :])
        nc.sync.dma_start(wv_sb[:, 0:4, :], wv_e.ap()[:, 0:4, :])
        nc.sync.dma_start(wv_sb[:, 4:, :], wv_e.ap()[:, 4:, :])
        nc.sync.dma_start(wq_sb[:, 0], wq_e.ap()[:, 0])
        nc.sync.dma_start(bq_sb[:], bq_e.ap())
        nc.sync.dma_start(xT_sb[1][:], xT_e.ap()[1])
        nc.sync.dma_start(wk_sb[:, 1], wk_e.ap()[:, 1])
        nc.sync.dma_start(wq_sb[:, 1], wq_e.ap()[:, 1])
        for q in range(2, NQS):
            nc.sync.dma_start(xT_sb[q][:], xT_e.ap()[q])
        nc.sync.dma_start(wo_sb[:], wo_e.ap())

        # K^T projection group: KT[128 (2 heads), m, s]
        def emit_kproj_group(m, q, tag="mm"):
            ps = psum.tile([128, SP], F32, tag=tag, name="kps")
            for c in range(NDC):
                nc.tensor.matmul(
                    ps[:], wk_sb[:, m, c, :],
                    xT_sb[q][:, c, :],
                    start=(c == 0), stop=(c == NDC - 1))
            nc.vector.tensor_scalar(
                KT_sb[:, m, q * SP:(q + 1) * SP], ps[:],
                bk_sb[:, m:m + 1], None, ADD)

        # V projection into augmented layout [s-chunk, 4*(64+1)];
        # the ones columns are set by a DVE memset afterwards
        def emit_vproj_group(sc, tag="mm"):
            q, si = divmod(sc, SI)
            ps = psum.tile([128, HA], F32, tag=tag, name="vps")
            for c in range(NDC):
                nc.tensor.matmul(
                    ps[:], xT_sb[q][:, c, si * 128:(si + 1) * 128],
                    wv_sb[:, c, :],
                    start=(c == 0), stop=(c == NDC - 1))
            nc.vector.tensor_copy(V_sb[:, sc, :], ps[:])
            vsc = V_sb[:, sc, :].rearrange("p (h x) -> p h x", x=HAUG)
            nc.vector.memset(vsc[:, :, DK:DK + 1], 1.0)

        # last-span W_o is two-pass: m0 partials land in these persistent
        # SBUF tiles mid-span, only m1 + add + DMA remain for the epilogue
        woa = [const.tile([128, SP], F32, name=f"woa{i}")
               for i in range(SI * (D // SP))]

        def make_qproj_parts(QTn, qsrc, m, nparts=2):
            cell = []
            step = NDC // nparts
            def part(p):
                def run():
                    if p == 0:
                        cell.append(psum.tile([128, SP], F32, tag="wo",
                                              name="qps"))
                    ps = cell[0]
                    for c in range(p * step, (p + 1) * step):
                        nc.tensor.matmul(
                            ps[:], wq_sb[:, m, c, :],
                            xT_sb[qsrc][:, c, :],
                            start=(c == 0), stop=(c == NDC - 1))
                    if p == nparts - 1:
                        nc.vector.tensor_scalar(
                            QTn[:, m, :], ps[:], bq_sb[:, m:m + 1], None, ADD)
                return run
            return [part(p) for p in range(nparts)]

        def emit_qproj_group(QTn, qsrc, m):
            for run in make_qproj_parts(QTn, qsrc, m, nparts=1):
                run()

        # logits + exp for a head PAIR: the two heads' K=64 matmuls run
        # concurrently on disjoint PE row-groups (rows 0:64 / 64:128).
        # Each key-chunk's PAIR shares one 2-bank psum tile (head A in
        # bank 0, head B in bank 1) so both matmuls become ready at the
        # same instant -- the Tile scheduler then keeps them adjacent and
        # the hardware overlaps them. One exp covers the pair.
        def emit_lg_exp_pair(QT, pair, scp, dve_js=()):
            m = pair
            lgs, mms, pts = [], [], []
            for j in range(2):
                sc = 2 * scp + j
                lg = psum.tile([128, 2, SP], F32, tag="mm", name="lg")
                mms.append(nc.tensor.matmul(
                    lg[:, 0, :],
                    KT_sb[0:64, m, sc * 128:(sc + 1) * 128],
                    QT[0:64, m, :], start=True, stop=True))
                mms.append(nc.tensor.matmul(
                    lg[:, 1, :],
                    KT_sb[64:128, m, sc * 128:(sc + 1) * 128],
                    QT[64:128, m, :], start=True, stop=True))
                lgs.append(lg)
            for j in range(2):
                pt = ptp.tile([128, 2, SP], BF16, name="pt")
                if j in dve_js:
                    # Schraudolph exp on DVE (offloads the ACT bottleneck)
                    nc.vector.tensor_scalar(
                        pt[:].bitcast(mybir.dt.int16), lgs[j][:],
                        S1K, S2K, MULT, ADD)
                else:
                    nc.scalar.activation(pt[:], lgs[j][:], EXP,
                                         scale=float(SCALE))
                pts.append(pt)
            return pts

        def emit_ctx(CT, cps_by_h, h, hi, scp, pts):
            if scp == 0:
                cps_by_h[h] = psum.tile([HAUG, SP], F32, tag="ctx",
                                        name="cps")
            cps = cps_by_h[h]
            for j in range(2):
                sc = 2 * scp + j
                nc.tensor.matmul(
                    cps[:], V_sb[:, sc, h * HAUG:(h + 1) * HAUG],
                    pts[j][:, hi, :],
                    start=(sc == 0), stop=(sc == NSC - 1))
            if scp == NSC // 2 - 1:
                return (h, cps)
            return None

        def emit_norm(CT, h, cps):
            # deferred: runs a bit later so nothing here sits at the
            # head of the PE queue. No PE instruction in this chain —
            # the partition broadcast runs on the (otherwise idle) gpsimd.
            m, r = divmod(h, 2)
            r *= 64
            sm = smp.tile([1, SP], F32, name="sm")
            nc.vector.tensor_copy(sm[:], cps[DK:DK + 1, :])
            rc = smp.tile([1, SP], F32, name="rc")
            nc.vector.reciprocal_approx_fast(rc[:], sm[:])
            bc = smp.tile([64, SP], F32, name="bc")
            nc.gpsimd.partition_broadcast(bc[:], rc[:])
            nc.vector.tensor_tensor(
                CT[r:r + 64, m, :], cps[0:DK, :], bc[:], MULT)

        def emit_fast_tail(CT, normA, normB):
            # epilogue-only: last pair's norms with the copies on the (idle)
            # ACT engine, then the final W_o pass pipelined per si-chunk so
            # PE matmuls overlap the remaining DVE multiplies.
            (hA, cpsA), (hB, cpsB) = normA, normB
            mA, rA = divmod(hA, 2)
            mB, rB = divmod(hB, 2)
            rA *= 64
            rB *= 64
            assert mA == mB == 1 and rA == 0 and rB == 64
            smA = smp.tile([1, SP], F32, name="smA")
            nc.vector.tensor_copy(smA[:], cpsA[DK:DK + 1, :])
            smB = smp.tile([1, SP], F32, name="smB")
            nc.vector.tensor_copy(smB[:], cpsB[DK:DK + 1, :])
            rcA = smp.tile([1, SP], F32, name="rcA")
            nc.vector.reciprocal_approx_fast(rcA[:], smA[:])
            rcB = smp.tile([1, SP], F32, name="rcB")
            nc.vector.reciprocal_approx_fast(rcB[:], smB[:])
            bcA = smp.tile([64, SP], F32, name="bcA")
            nc.gpsimd.partition_broadcast(bcA[:], rcA[:])
            bcB = smp.tile([64, SP], F32, name="bcB")
            nc.gpsimd.partition_broadcast(bcB[:], rcB[:])
            for si in range(SI):
                sl = slice(si * 128, (si + 1) * 128)
                nc.vector.tensor_tensor(
                    CT[0:64, 1, sl], cpsA[0:DK, sl], bcA[:, sl], MULT)
                nc.vector.tensor_tensor(
                    CT[64:128, 1, sl], cpsB[0:DK, sl], bcB[:, sl], MULT)
                sc = SI * (NQS - 1) + si
                for dh in range(D // SP):
                    i = si * (D // SP) + dh
                    po = psum.tile([128, SP], F32, tag="wo", name="po")
                    nc.tensor.matmul(
                        po[:], CT[:, 1, sl],
                        wo_sb[:, 1, dh * SP:(dh + 1) * SP],
                        start=True, stop=True)
                    ob = obp.tile([128, SP], F32, name="ob")
                    nc.vector.tensor_tensor(ob[:], woa[i][:], po[:], ADD)
                    nc.sync.dma_start(
                        out_e.ap()[sc * 128:(sc + 1) * 128,
                                   dh * SP:(dh + 1) * SP], ob[:])

        def make_wo_ops(q, CT, split_copies=False):
            # each (si, dh) group split into two single-matmul halves so the
            # interleave never adds more than one extra matmul per period
            ops = []
            for si in range(SI):
                sc = SI * q + si
                for dh in range(D // SP):
                    on_act = split_copies and (si * (D // SP) + dh) % 2 == 1
                    cell = []
                    def half_a(si=si, dh=dh, CT=CT, cell=cell):
                        cell.append(psum.tile([128, SP], F32, tag="wo",
                                              name="po"))
                        nc.tensor.matmul(
                            cell[0][:], CT[:, 0, si * 128:(si + 1) * 128],
                            wo_sb[:, 0, dh * SP:(dh + 1) * SP],
                            start=True, stop=False)
                    def half_b(si=si, sc=sc, dh=dh, CT=CT, cell=cell,
                               on_act=on_act):
                        po = cell[0]
                        nc.tensor.matmul(
                            po[:], CT[:, 1, si * 128:(si + 1) * 128],
                            wo_sb[:, 1, dh * SP:(dh + 1) * SP],
                            start=False, stop=True)
                        ob = obp.tile([128, SP], F32, name="ob")
                        if on_act:
                            nc.scalar.activation(
                                ob[:], po[:],
                                mybir.ActivationFunctionType.Copy)
                        else:
                            nc.vector.tensor_copy(ob[:], po[:])
                        nc.sync.dma_start(
                            out_e.ap()[sc * 128:(sc + 1) * 128,
                                       dh * SP:(dh + 1) * SP], ob[:])
                    ops.append(half_a)
                    ops.append(half_b)
            return ops

        # prologue: only what span-0's FIRST pair-job needs: K^T m=0 for
        # key chunks 0..3, Q^T m=0, V chunks 0..1. Everything else (K^T
        # m=0 q1-3, K^T m=1, V 2..15, Q^T m=1) is deadline-scheduled into
        # span 0's job stream: the logits of (pair, scp) only touch KT
        # chunks 2scp..2scp+1 and ctx only needs V chunk 2k when job k+1
        # drains, so the projection work can spread across the whole span
        # instead of front-loading.
        PJPS = (H // GPB // 2) * (NSC // 2)   # pair-jobs per span (16)
        QT_t = {0: qpool.tile([128, 2, SP], BF16, name="QT")}
        emit_kproj_group(0, 0)
        emit_qproj_group(QT_t[0], 0, 0)
        emit_vproj_group(0)
        emit_vproj_group(1)
        CT_t = {}
        cps_t = {}
        pend = []                       # [(q, pair, scp, pts)]
        wo_queue = []
        # (deadline_job, op): op must be EMITTED at a job <= deadline_job.
        # All run on wo-tag psum slots (no W_o traffic exists in span 0).
        fill_queue = []
        for kq in range(1, NQS):
            fill_queue.append((2 * kq - 1,
                               lambda q=kq: emit_kproj_group(0, q, tag="wo")))
        for sc in range(2, NSC):
            fill_queue.append((sc // 2,
                               lambda sc=sc: emit_vproj_group(sc, tag="wo")))
        fill_queue.append((6, lambda: emit_qproj_group(QT_t[0], 0, 1)))
        for kq in range(NQS):
            fill_queue.append((7 + 2 * kq,
                               lambda q=kq: emit_kproj_group(1, q, tag="wo")))
        fill_queue.sort(key=lambda e: e[0])

        def drain_pend(gidx, limit=1, inline_norms=False):
            collected = []
            while len(pend) > limit:
                eq, epair, escp, epts = pend.pop(0)
                for hi in range(2):
                    nrm = emit_ctx(CT_t[eq], cps_t[eq], 2 * epair + hi, hi,
                                   escp, epts)
                    if nrm is not None:
                        if inline_norms:
                            emit_norm(CT_t[eq], *nrm)
                        else:
                            collected.append((eq, nrm))
            return collected

        # spans 1+ offload 10 of their 32 chunk-exps to the DVE so the ACT
        # engine (the per-span bottleneck in spans 1-3) drops below the
        # span's PE time; the last two jobs of each pair stay on ACT so the
        # span-final norm chain doesn't queue behind DVE work.
        def dve_pattern(q, scp):
            if q == 0:
                return ()
            if scp in (1, 2, 3, 5):
                return (0,)
            if scp == 4:
                return (1,)
            return ()

        pending_norms = []
        for q in range(NQS):
            CT_t[q] = cpool.tile([128, 2, SP], BF16, name="CT")
            cps_t[q] = {}
            qpart_queue = []
            if q + 1 < NQS:
                QT_t[q + 1] = qpool.tile([128, 2, SP], BF16, name="QT")
                qpart_queue = (make_qproj_parts(QT_t[q + 1], q + 1, 0,
                                                nparts=NDC)
                               + make_qproj_parts(QT_t[q + 1], q + 1, 1,
                                                  nparts=NDC))

            passa = []
            if q == NQS - 1:
                def mk_passa(i, si, dh, CT=CT_t[q]):
                    def run():
                        po = psum.tile([128, SP], F32, tag="wo", name="po")
                        nc.tensor.matmul(
                            po[:], CT[:, 0, si * 128:(si + 1) * 128],
                            wo_sb[:, 0, dh * SP:(dh + 1) * SP],
                            start=True, stop=True)
                        nc.vector.tensor_copy(woa[i][:], po[:])
                    return run
                passa = [mk_passa(si * (D // SP) + dh, si, dh)
                         for si in range(SI) for dh in range(D // SP)]

            for pjob in range(PJPS):
                gidx = q * PJPS + pjob
                pair, scp = divmod(pjob, NSC // 2)
                # norms collected from the previous job's drain run first:
                # they free the ctx psum slots this job's drain reallocates
                for eq, nrm in pending_norms:
                    emit_norm(CT_t[eq], *nrm)
                pending_norms = []
                pts = emit_lg_exp_pair(QT_t[q], pair, scp,
                                       dve_js=dve_pattern(q, scp))
                pend.append((q, pair, scp, pts))
                # span-0 projection filler: everything past its deadline,
                # topped up to two ops to keep the PE streaming
                npop = 0
                while fill_queue and (fill_queue[0][0] <= pjob or npop < 2):
                    if fill_queue[0][0] > pjob and npop >= 2:
                        break
                    fill_queue.pop(0)[1]()
                    npop += 1
                # 32 W_o halves per span over jobs 2..15
                if pjob >= 2:
                    nwo = 3 if pjob < 6 else 2
                    for _ in range(nwo):
                        if wo_queue:
                            wo_queue.pop(0)()
                if 8 <= pjob < 16:
                    for _ in range(2):
                        if qpart_queue:
                            qpart_queue.pop(0)()
                if passa and pjob >= 10:
                    for _ in range(2):
                        if passa:
                            passa.pop(0)()
                if pjob == PJPS - 1:
                    # span end: drain everything now (norms inline) so the
                    # old span's CT is fully written before the next span's
                    # jobs emit W_o readers of it
                    if q < NQS - 1:
                        drain_pend(gidx, limit=0, inline_norms=True)
                        pending_norms = []
                    else:
                        pending_norms = drain_pend(gidx, limit=0)
                else:
                    # first ctx of a span lags 3 jobs (pts pool is 8 deep)
                    # so no ctx matmul blocked on the old span's norm chain
                    # ever heads the FIFO PE queue
                    pending_norms = drain_pend(gidx, limit=3 if pjob <= 2
                                               else 1)
            if q < NQS - 1:
                wo_queue.extend(make_wo_ops(q, CT_t[q]))

        # epilogue: drain the pipeline. The last pair's two norms and the
        # final W_o pass run as one per-si pipelined fast tail; leftover
        # pass-a matmuls keep the PE warm underneath the norm latency.
        gidx = NQS * PJPS
        last_norms = pending_norms + drain_pend(gidx, limit=0,
                                                inline_norms=False)
        pending_norms = []
        for op in wo_queue:
            op()
        for eq, nrm in last_norms[:-2]:
            emit_norm(CT_t[eq], *nrm)
        for op in passa:
            op()
        CTl = CT_t[NQS - 1]
        if len(last_norms) >= 2 and last_norms[-2][1][0] == 2:
            emit_fast_tail(CTl, last_norms[-2][1], last_norms[-1][1])
        else:
            for eq, nrm in last_norms[-2:]:
                emit_norm(CT_t[eq], *nrm)
            for si in range(SI):
                sc = SI * (NQS - 1) + si
                for dh in range(D // SP):
                    i = si * (D // SP) + dh
                    po = psum.tile([128, SP], F32, tag="wo", name="po")
                    nc.tensor.matmul(
                        po[:], CTl[:, 1, si * 128:(si + 1) * 128],
                        wo_sb[:, 1, dh * SP:(dh + 1) * SP],
                        start=True, stop=True)
                    ob = obp.tile([128, SP], F32, name="ob")
                    nc.vector.tensor_tensor(ob[:], woa[i][:], po[:], ADD)
                    nc.sync.dma_start(
                        out_e.ap()[sc * 128:(sc + 1) * 128,
                                   dh * SP:(dh + 1) * SP], ob[:])

    nc.compile()
    return nc


_NC_CACHE = {}


def get_nc(S):
    if S not in _NC_CACHE:
        _NC_CACHE[S] = build(S)
    return _NC_CACHE[S]


def make_in_maps(x, W_q, b_q, W_k, b_k, W_v, b_v, W_o, b_o):
    B, S, _ = x.shape
    bf = ml_dtypes.bfloat16
    in_maps = []
    for core in range(NCORES):
        b, g = divmod(core, GPB)
        sl = slice(HD * g, HD * (g + 1))
        wv_aug = np.zeros((D, HPG * HAUG), np.float32)
        wv_block = np.asarray(W_v[:, sl]).reshape(D, HPG, DK)
        wv_aug.reshape(D, HPG, HAUG)[:, :, :DK] = wv_block
        def wtile_m(w):
            # [D, 256] -> [128, 2, D//128, 128] m-major chunk layout
            return np.ascontiguousarray(
                np.asarray(w).reshape(D // 128, 128, 2, 128)
                .transpose(1, 2, 0, 3))
        def wtile(w):
            # [D, N] -> [128, D//128, N] partition-major chunk layout
            return np.ascontiguousarray(
                np.asarray(w).reshape(D // 128, 128, -1).transpose(1, 0, 2))
        in_maps.append({
            "xT": np.ascontiguousarray(
                np.asarray(x[b]).T.reshape(D // 128, 128, S // SP, SP)
                .transpose(2, 1, 0, 3)).astype(bf),
            "wq": wtile_m(W_q[:, sl]).astype(bf),
            "wk": wtile_m(W_k[:, sl]).astype(bf),
            "wv": wtile(wv_aug).astype(bf),
            "wo": np.ascontiguousarray(
                np.asarray(W_o[sl, :]).reshape(2, 128, D)
                .transpose(1, 0, 2)).astype(bf),
            "bq": np.ascontiguousarray(
                np.asarray(b_q[sl]).reshape(2, 128).T).astype(np.float32),
            "bk": np.ascontiguousarray(
                np.asarray(b_k[sl]).reshape(2, 128).T).astype(np.float32),
        })
    return in_maps


def unshard(results, x, W_o, b_v, b_o):
    B, S, _ = x.shape
    out = np.zeros((B, S, D), np.float32)
    for core in range(NCORES):
        b = core // GPB
        out[b] += results[core]["out"]
    const = np.asarray(b_v).astype(np.float64) @ np.asarray(W_o).astype(np.float64)
    const += np.asarray(b_o).astype(np.float64)
    out += const.astype(np.float32)[None, None, :]
    return out


def run(inputs, trace=False):
    x = np.asarray(inputs["x"])
    nc = get_nc(x.shape[1])
    in_maps = make_in_maps(
        x, inputs["W_q"], inputs["b_q"], inputs["W_k"], inputs["b_k"],
        inputs["W_v"], inputs["b_v"], inputs["W_o"], inputs["b_o"])
    def attempt():
        res = run_bass_kernel_spmd(
            nc, in_maps, core_ids=list(range(NCORES)), trace=trace)
        # force materialization here: PJRT surfaces device errors lazily
        res.results = [{k: np.asarray(v) for k, v in r.items()}
                       for r in res.results]
        return res
    try:
        res = attempt()
    except Exception:
        # transient device errors (e.g. NRT_EXEC_UNIT_UNRECOVERABLE) clear
        # on re-execution of the same NEFF
        res = attempt()
    out = unshard(res.results, x, inputs["W_o"], inputs["b_v"], inputs["b_o"])
    return out, res


def kernel(**inputs):
    out, _ = run(inputs, trace=False)
    return out



# revision 51
# speedup vs baseline: 1.0108x; 1.0108x over previous
"""Multi-head attention (B=2, S=2048, D=1024, H=16) on 8 TRN2 NeuronCores.

Sharding: tensor-parallel over heads x data-parallel over batch.
Core c handles batch b = c//4, head group g = c%4 (4 heads, 256 cols).
W_q/W_k/W_v are split column-wise per group, W_o row-wise; each core
produces a partial [S, D] output, reduced on the host (the W_o
contraction is a pure sum over head groups; b_v/b_o folded in on host).

Device kernel (per core), all matmuls bf16 with fp32 PSUM accumulation:
  - K^T, Q^T projections in transposed layout [dk*2, S] (lhsT = W cols,
    rhs = x^T), V in natural layout [S, dk*4+ones] (lhsT = x^T chunks).
  - scores computed transposed: ST[k,q] = (K^T chunk)^T-matmul vs Q^T.
    The two heads of a pair (rows 0:64 / 64:128 of KT/QT) are emitted
    interleaved so their K=64 matmuls run CONCURRENTLY on disjoint PE
    row-groups (tile_position auto-derives from base_partition) --
    halving the logits phase vs sequential per-head emission.
  - softmax without max-subtraction (logits are O(5) here, exp is safe):
    exp on ACT straight out of PSUM with scale=1/sqrt(dk).
  - ctx^T[dk+1, q] accumulated over k-chunks with an ones-augmented V
    (row dk = softmax denominators), normalized via DVE with a gpsimd
    partition-broadcast of the reciprocals.
  - out partial = ctx^T-chunks @ W_o rows, accumulated over the 2
    128-row chunks of the group's 256 W_o rows.
"""

import numpy as np
import ml_dtypes
from contextlib import ExitStack

import concourse.bass as bass
import concourse.tile as tile
from concourse import bacc, mybir
from concourse.bass_utils import run_bass_kernel_spmd

BF16 = mybir.dt.bfloat16
F32 = mybir.dt.float32

D = 1024            # model dim
H = 16              # heads
DK = 64             # head dim
NCORES = 8
GPB = 4             # head groups per batch (= cores per batch)
HPG = H // GPB      # 4 heads per core
HD = HPG * DK       # 256 cols per group
HAUG = DK + 1       # 65: head block width in augmented-V layout
SP = 512            # q-span / free-dim tile
SCALE = 1.0 / np.sqrt(DK)
# Schraudolph exp on DVE: i16 = rne(logit*S1K + S2K); bitcast int16->bf16
# approximates exp(logit*SCALE) with ~+-3.3% sawtooth error that cancels
# in the softmax normalization (denominator sums the same approx values).
SCH_C = 0.0440
S1K = float(SCALE * np.log2(np.e) * 128.0)
S2K = float((127.0 - SCH_C) * 128.0)


def build(S):
    NQS = S // SP       # q spans
    NSC = S // 128      # sequence chunks (k side)
    NDC = D // 128      # model-dim chunks
    SI = SP // 128      # s-chunks per q-span

    nc = bacc.Bacc("TRN2", target_bir_lowering=False, debug=False)
    HA = HPG * HAUG     # 260: augmented V width
    xT_e = nc.dram_tensor("xT", [S // SP, 128, D // 128, SP], BF16, kind="ExternalInput")
    # m-major weight layout: [128, m, c, 128] so each m-half is one
    # contiguous DMA (m=0 is all the prologue needs)
    wq_e = nc.dram_tensor("wq", [128, 2, D // 128, 128], BF16, kind="ExternalInput")
    wk_e = nc.dram_tensor("wk", [128, 2, D // 128, 128], BF16, kind="ExternalInput")
    wv_e = nc.dram_tensor("wv", [128, D // 128, HA], BF16, kind="ExternalInput")
    wo_e = nc.dram_tensor("wo", [128, 2, D], BF16, kind="ExternalInput")
    bq_e = nc.dram_tensor("bq", [128, 2], F32, kind="ExternalInput")
    bk_e = nc.dram_tensor("bk", [128, 2], F32, kind="ExternalInput")
    out_e = nc.dram_tensor("out", [S, D], F32, kind="ExternalOutput")

    ADD = mybir.AluOpType.add
    MULT = mybir.AluOpType.mult
    EXP = mybir.ActivationFunctionType.Exp

    with tile.TileContext(nc) as tc, ExitStack() as ctx:
        const = ctx.enter_context(tc.tile_pool(name="const", bufs=1))
        qpool = ctx.enter_context(tc.tile_pool(name="qpool", bufs=2))
        cpool = ctx.enter_context(tc.tile_pool(name="cpool", bufs=2))
        ptp = ctx.enter_context(tc.tile_pool(name="ptp", bufs=8))
        obp = ctx.enter_context(tc.tile_pool(name="obp", bufs=4))
        smp = ctx.enter_context(tc.tile_pool(name="smp", bufs=4))
        psum = ctx.enter_context(tc.tile_pool(name="psum", bufs=2, space="PSUM"))

        wq_sb = const.tile([128, 2, NDC, 128], BF16, name="wq_sb")
        wk_sb = const.tile([128, 2, NDC, 128], BF16, name="wk_sb")
        wv_sb = const.tile([128, NDC, HA], BF16, name="wv_sb")
        wo_sb = const.tile([128, 2, D], BF16, name="wo_sb")
        bq_sb = const.tile([128, 2], F32, name="bq_sb")
        bk_sb = const.tile([128, 2], F32, name="bk_sb")
        xT_sb = [const.tile([128, NDC, SP], BF16, name=f"xT{q}") for q in range(NQS)]
        KT_sb = const.tile([128, 2, S], BF16, name="KT_sb")
        V_sb = const.tile([128, NSC, HPG * HAUG], BF16, name="V_sb")

        # PE warm-up: ~20 dummy matmuls on never-written SBUF scratch with
        # no DMA dependency -- they start at t~0 and keep the HAM activity
        # window non-idle through the ~11us DMA prologue, so the first real
        # kproj matmuls run at the full 2.4 GHz instead of the cold 1.2.
        warm = const.tile([128, SP], BF16, name="warm")
        nc.vector.memset(warm[:], 0.0)
        wps = psum.tile([128, SP], F32, tag="mm", name="wps")
        for w in range(8):
            nc.tensor.matmul(wps[:], warm[:, 0:128], warm[:],
                             start=(w == 0), stop=(w == 7))

        def emit_dummies(n):
            # throwaway matmuls on the scratch tile into unread wo-tag
            # psum: keeps the HAM activity window non-idle across known
            # PE-idle gaps (span boundaries, tail norm chain) so the
            # following real matmuls run at 2.4 GHz instead of 1.2
            for _ in range(n):
                dps = psum.tile([128, SP], F32, tag="wo", name="dps")
                nc.tensor.matmul(dps[:], warm[:, 0:128], warm[:],
                                 start=True, stop=True)

        # input DMAs, in need-order. The first few land on distinct DMA
        # queues and run in parallel; fine granularity on the span-0
        # pieces lets the first kproj matmuls start as early as possible.
        nc.sync.dma_start(wk_sb[:, 0, 0:4, :], wk_e.ap()[:, 0, 0:4, :])
        nc.sync.dma_start(xT_sb[0][:, 0:2, :], xT_e.ap()[0, :, 0:2, :])
        nc.sync.dma_start(bk_sb[:], bk_e.ap())
        nc.sync.dma_start(wk_sb[:, 0, 4:, :], wk_e.ap()[:, 0, 4:, :])
        nc.sync.dma_start(xT_sb[0][:, 2:4, :], xT_e.ap()[0, :, 2:4, :])
        nc.sync.dma_start(xT_sb[0][:, 4:6, :], xT_e.ap()[0, :, 4:6, :])
        nc.sync.dma_start(xT_sb[0][:, 6:, :], xT_e.ap()[0, :, 6:, :])
        nc.sync.dma_start(wv_sb[:, 0:4, :], wv_e.ap()[:, 0:4, :])
        nc.sync.dma_start(wv_sb[:, 4:, :], wv_e.ap()[:, 4:, :])
        nc.sync.dma_start(wq_sb[:, 0], wq_e.ap()[:, 0])
        nc.sync.dma_start(bq_sb[:], bq_e.ap())
        nc.sync.dma_start(xT_sb[1][:], xT_e.ap()[1])
        nc.sync.dma_start(wk_sb[:, 1], wk_e.ap()[:, 1])
        nc.sync.dma_start(wq_sb[:, 1], wq_e.ap()[:, 1])
        for q in range(2, NQS):
            nc.sync.dma_start(xT_sb[q][:], xT_e.ap()[q])
        nc.sync.dma_start(wo_sb[:], wo_e.ap())

        # K^T projection group: KT[128 (2 heads), m, s]
        def emit_kproj_group(m, q, tag="mm"):
            ps = psum.tile([128, SP], F32, tag=tag, name="kps")
            for c in range(NDC):
                nc.tensor.matmul(
                    ps[:], wk_sb[:, m, c, :],
                    xT_sb[q][:, c, :],
                    start=(c == 0), stop=(c == NDC - 1))
            nc.vector.tensor_scalar(
                KT_sb[:, m, q * SP:(q + 1) * SP], ps[:],
                bk_sb[:, m:m + 1], None, ADD)

        # V projection into augmented layout [s-chunk, 4*(64+1)];
        # the ones columns are set by a DVE memset afterwards
        def emit_vproj_group(sc, tag="mm"):
            q, si = divmod(sc, SI)
            ps = psum.tile([128, HA], F32, tag=tag, name="vps")
            for c in range(NDC):
                nc.tensor.matmul(
                    ps[:], xT_sb[q][:, c, si * 128:(si + 1) * 128],
                    wv_sb[:, c, :],
                    start=(c == 0), stop=(c == NDC - 1))
            nc.vector.tensor_copy(V_sb[:, sc, :], ps[:])
            vsc = V_sb[:, sc, :].rearrange("p (h x) -> p h x", x=HAUG)
            nc.vector.memset(vsc[:, :, DK:DK + 1], 1.0)

        # last-span W_o is two-pass: m0 partials land in these persistent
        # SBUF tiles mid-span, only m1 + add + DMA remain for the epilogue
        woa = [const.tile([128, SP], F32, name=f"woa{i}")
               for i in range(SI * (D // SP))]

        def make_qproj_parts(QTn, qsrc, m, nparts=2):
            cell = []
            step = NDC // nparts
            def part(p):
                def run():
                    if p == 0:
                        cell.append(psum.tile([128, SP], F32, tag="wo",
                                              name="qps"))
                    ps = cell[0]
                    for c in range(p * step, (p + 1) * step):
                        nc.tensor.matmul(
                            ps[:], wq_sb[:, m, c, :],
                            xT_sb[qsrc][:, c, :],
                            start=(c == 0), stop=(c == NDC - 1))
                    if p == nparts - 1:
                        nc.vector.tensor_scalar(
                            QTn[:, m, :], ps[:], bq_sb[:, m:m + 1], None, ADD)
                return run
            return [part(p) for p in range(nparts)]

        def emit_qproj_group(QTn, qsrc, m):
            for run in make_qproj_parts(QTn, qsrc, m, nparts=1):
                run()

        # logits + exp for a head PAIR: the two heads' K=64 matmuls run
        # concurrently on disjoint PE row-groups (rows 0:64 / 64:128).
        # Each key-chunk's PAIR shares one 2-bank psum tile (head A in
        # bank 0, head B in bank 1) so both matmuls become ready at the
        # same instant -- the Tile scheduler then keeps them adjacent and
        # the hardware overlaps them. One exp covers the pair.
        def emit_lg_exp_pair(QT, pair, scp, dve_js=()):
            m = pair
            lgs, mms, pts = [], [], []
            for j in range(2):
                sc = 2 * scp + j
                lg = psum.tile([128, 2, SP], F32, tag="mm", name="lg")
                mms.append(nc.tensor.matmul(
                    lg[:, 0, :],
                    KT_sb[0:64, m, sc * 128:(sc + 1) * 128],
                    QT[0:64, m, :], start=True, stop=True))
                mms.append(nc.tensor.matmul(
                    lg[:, 1, :],
                    KT_sb[64:128, m, sc * 128:(sc + 1) * 128],
                    QT[64:128, m, :], start=True, stop=True))
                lgs.append(lg)
            for j in range(2):
                pt = ptp.tile([128, 2, SP], BF16, name="pt")
                if j in dve_js:
                    # Schraudolph exp on DVE (offloads the ACT bottleneck)
                    nc.vector.tensor_scalar(
                        pt[:].bitcast(mybir.dt.int16), lgs[j][:],
                        S1K, S2K, MULT, ADD)
                else:
                    nc.scalar.activation(pt[:], lgs[j][:], EXP,
                                         scale=float(SCALE))
                pts.append(pt)
            return pts

        def emit_ctx(CT, cps_by_h, h, hi, scp, pts):
            if scp == 0:
                cps_by_h[h] = psum.tile([HAUG, SP], F32, tag="ctx",
                                        name="cps")
            cps = cps_by_h[h]
            for j in range(2):
                sc = 2 * scp + j
                nc.tensor.matmul(
                    cps[:], V_sb[:, sc, h * HAUG:(h + 1) * HAUG],
                    pts[j][:, hi, :],
                    start=(sc == 0), stop=(sc == NSC - 1))
            if scp == NSC // 2 - 1:
                return (h, cps)
            return None

        def emit_norm(CT, h, cps):
            # deferred: runs a bit later so nothing here sits at the
            # head of the PE queue. No PE instruction in this chain —
            # the partition broadcast runs on the (otherwise idle) gpsimd.
            m, r = divmod(h, 2)
            r *= 64
            sm = smp.tile([1, SP], F32, name="sm")
            nc.vector.tensor_copy(sm[:], cps[DK:DK + 1, :])
            rc = smp.tile([1, SP], F32, name="rc")
            nc.vector.reciprocal_approx_fast(rc[:], sm[:])
            bc = smp.tile([64, SP], F32, name="bc")
            nc.gpsimd.partition_broadcast(bc[:], rc[:])
            nc.vector.tensor_tensor(
                CT[r:r + 64, m, :], cps[0:DK, :], bc[:], MULT)

        def emit_fast_tail(CT, normA, normB):
            # epilogue-only: last pair's norms with the copies on the (idle)
            # ACT engine, then the final W_o pass pipelined per si-chunk so
            # PE matmuls overlap the remaining DVE multiplies.
            (hA, cpsA), (hB, cpsB) = normA, normB
            mA, rA = divmod(hA, 2)
            mB, rB = divmod(hB, 2)
            rA *= 64
            rB *= 64
            assert mA == mB == 1 and rA == 0 and rB == 64
            smA = smp.tile([1, SP], F32, name="smA")
            nc.vector.tensor_copy(smA[:], cpsA[DK:DK + 1, :])
            smB = smp.tile([1, SP], F32, name="smB")
            nc.vector.tensor_copy(smB[:], cpsB[DK:DK + 1, :])
            rcA = smp.tile([1, SP], F32, name="rcA")
            nc.vector.reciprocal_approx_fast(rcA[:], smA[:])
            rcB = smp.tile([1, SP], F32, name="rcB")
            nc.vector.reciprocal_approx_fast(rcB[:], smB[:])
            bcA = smp.tile([64, SP], F32, name="bcA")
            nc.gpsimd.partition_broadcast(bcA[:], rcA[:])
            bcB = smp.tile([64, SP], F32, name="bcB")
            nc.gpsimd.partition_broadcast(bcB[:], rcB[:])
            emit_dummies(10)
            for si in range(SI):
                sl = slice(si * 128, (si + 1) * 128)
                nc.vector.tensor_tensor(
                    CT[0:64, 1, sl], cpsA[0:DK, sl], bcA[:, sl], MULT)
                nc.vector.tensor_tensor(
                    CT[64:128, 1, sl], cpsB[0:DK, sl], bcB[:, sl], MULT)
                sc = SI * (NQS - 1) + si
                for dh in range(D // SP):
                    i = si * (D // SP) + dh
                    po = psum.tile([128, SP], F32, tag="wo", name="po")
                    nc.tensor.matmul(
                        po[:], CT[:, 1, sl],
                        wo_sb[:, 1, dh * SP:(dh + 1) * SP],
                        start=True, stop=True)
                    ob = obp.tile([128, SP], F32, name="ob")
                    nc.vector.tensor_tensor(ob[:], woa[i][:], po[:], ADD)
                    nc.sync.dma_start(
                        out_e.ap()[sc * 128:(sc + 1) * 128,
                                   dh * SP:(dh + 1) * SP], ob[:])

        def make_wo_ops(q, CT, split_copies=False):
            # each (si, dh) group split into two single-matmul halves so the
            # interleave never adds more than one extra matmul per period
            ops = []
            for si in range(SI):
                sc = SI * q + si
                for dh in range(D // SP):
                    on_act = split_copies and (si * (D // SP) + dh) % 2 == 1
                    cell = []
                    def half_a(si=si, dh=dh, CT=CT, cell=cell):
                        cell.append(psum.tile([128, SP], F32, tag="wo",
                                              name="po"))
                        nc.tensor.matmul(
                            cell[0][:], CT[:, 0, si * 128:(si + 1) * 128],
                            wo_sb[:, 0, dh * SP:(dh + 1) * SP],
                            start=True, stop=False)
                    def half_b(si=si, sc=sc, dh=dh, CT=CT, cell=cell,
                               on_act=on_act):
                        po = cell[0]
                        nc.tensor.matmul(
                            po[:], CT[:, 1, si * 128:(si + 1) * 128],
                            wo_sb[:, 1, dh * SP:(dh + 1) * SP],
                            start=False, stop=True)
                        ob = obp.tile([128, SP], F32, name="ob")
                        if on_act:
                            nc.scalar.activation(
                                ob[:], po[:],
                                mybir.ActivationFunctionType.Copy)
                        else:
                            nc.vector.tensor_copy(ob[:], po[:])
                        nc.sync.dma_start(
                            out_e.ap()[sc * 128:(sc + 1) * 128,
                                       dh * SP:(dh + 1) * SP], ob[:])
                    ops.append(half_a)
                    ops.append(half_b)
            return ops

        # prologue: only what span-0's FIRST pair-job needs: K^T m=0 for
        # key chunks 0..3, Q^T m=0, V chunks 0..1. Everything else (K^T
        # m=0 q1-3, K^T m=1, V 2..15, Q^T m=1) is deadline-scheduled into
        # span 0's job stream: the logits of (pair, scp) only touch KT
        # chunks 2scp..2scp+1 and ctx only needs V chunk 2k when job k+1
        # drains, so the projection work can spread across the whole span
        # instead of front-loading.
        PJPS = (H // GPB // 2) * (NSC // 2)   # pair-jobs per span (16)
        QT_t = {0: qpool.tile([128, 2, SP], BF16, name="QT")}
        emit_kproj_group(0, 0)
        emit_qproj_group(QT_t[0], 0, 0)
        emit_vproj_group(0)
        emit_vproj_group(1)
        CT_t = {}
        cps_t = {}
        pend = []                       # [(q, pair, scp, pts)]
        wo_queue = []
        # (deadline_job, op): op must be EMITTED at a job <= deadline_job.
        # All run on wo-tag psum slots (no W_o traffic exists in span 0).
        fill_queue = []
        for kq in range(1, NQS):
            fill_queue.append((2 * kq - 1,
                               lambda q=kq: emit_kproj_group(0, q, tag="wo")))
        for sc in range(2, NSC):
            fill_queue.append((sc // 2,
                               lambda sc=sc: emit_vproj_group(sc, tag="wo")))
        fill_queue.append((6, lambda: emit_qproj_group(QT_t[0], 0, 1)))
        for kq in range(NQS):
            fill_queue.append((7 + 2 * kq,
                               lambda q=kq: emit_kproj_group(1, q, tag="wo")))
        fill_queue.sort(key=lambda e: e[0])

        def drain_pend(gidx, limit=1, inline_norms=False):
            collected = []
            while len(pend) > limit:
                eq, epair, escp, epts = pend.pop(0)
                for hi in range(2):
                    nrm = emit_ctx(CT_t[eq], cps_t[eq], 2 * epair + hi, hi,
                                   escp, epts)
                    if nrm is not None:
                        if inline_norms:
                            emit_norm(CT_t[eq], *nrm)
                        else:
                            collected.append((eq, nrm))
            return collected

        # spans 1+ offload 10 of their 32 chunk-exps to the DVE so the ACT
        # engine (the per-span bottleneck in spans 1-3) drops below the
        # span's PE time; the last two jobs of each pair stay on ACT so the
        # span-final norm chain doesn't queue behind DVE work.
        def dve_pattern(q, scp):
            if q == 0:
                return ()
            if scp in (1, 2, 3, 5):
                return (0,)
            if scp == 4:
                return (1,)
            return ()

        pending_norms = []
        for q in range(NQS):
            CT_t[q] = cpool.tile([128, 2, SP], BF16, name="CT")
            cps_t[q] = {}
            qpart_queue = []
            if q + 1 < NQS:
                QT_t[q + 1] = qpool.tile([128, 2, SP], BF16, name="QT")
                qpart_queue = (make_qproj_parts(QT_t[q + 1], q + 1, 0,
                                                nparts=NDC)
                               + make_qproj_parts(QT_t[q + 1], q + 1, 1,
                                                  nparts=NDC))

            passa = []
            if q == NQS - 1:
                def mk_passa(i, si, dh, CT=CT_t[q]):
                    def run():
                        po = psum.tile([128, SP], F32, tag="wo", name="po")
                        nc.tensor.matmul(
                            po[:], CT[:, 0, si * 128:(si + 1) * 128],
                            wo_sb[:, 0, dh * SP:(dh + 1) * SP],
                            start=True, stop=True)
                        nc.vector.tensor_copy(woa[i][:], po[:])
                    return run
                passa = [mk_passa(si * (D // SP) + dh, si, dh)
                         for si in range(SI) for dh in range(D // SP)]

            for pjob in range(PJPS):
                gidx = q * PJPS + pjob
                pair, scp = divmod(pjob, NSC // 2)
                # norms collected from the previous job's drain run first:
                # they free the ctx psum slots this job's drain reallocates
                for eq, nrm in pending_norms:
                    emit_norm(CT_t[eq], *nrm)
                pending_norms = []
                pts = emit_lg_exp_pair(QT_t[q], pair, scp,
                                       dve_js=dve_pattern(q, scp))
                pend.append((q, pair, scp, pts))
                # span-0 projection filler: everything past its deadline,
                # topped up to two ops to keep the PE streaming
                npop = 0
                while fill_queue and (fill_queue[0][0] <= pjob or npop < 2):
                    if fill_queue[0][0] > pjob and npop >= 2:
                        break
                    fill_queue.pop(0)[1]()
                    npop += 1
                # 32 W_o halves per span over jobs 2..15
                if pjob >= 2:
                    nwo = 3 if pjob < 6 else 2
                    for _ in range(nwo):
                        if wo_queue:
                            wo_queue.pop(0)()
                if 8 <= pjob < 16:
                    for _ in range(2):
                        if qpart_queue:
                            qpart_queue.pop(0)()
                if q >= 1 and pjob <= 2:
                    emit_dummies(2)
                if passa and pjob >= 10:
                    for _ in range(2):
                        if passa:
                            passa.pop(0)()
                if pjob == PJPS - 1:
                    # span end: drain everything now (norms inline) so the
                    # old span's CT is fully written before the next span's
                    # jobs emit W_o readers of it
                    if q < NQS - 1:
                        drain_pend(gidx, limit=0, inline_norms=True)
                        pending_norms = []
                    else:
                        pending_norms = drain_pend(gidx, limit=0)
                else:
                    # first ctx of a span lags 3 jobs (pts pool is 8 deep)
                    # so no ctx matmul blocked on the old span's norm chain
                    # ever heads the FIFO PE queue
                    pending_norms = drain_pend(gidx, limit=3 if pjob <= 2
                                               else 1)
            if q < NQS - 1:
                wo_queue.extend(make_wo_ops(q, CT_t[q]))

        # epilogue: drain the pipeline. The last pair's two norms and the
        # final W_o pass run as one per-si pipelined fast tail; leftover
        # pass-a matmuls keep the PE warm underneath the norm latency.
        gidx = NQS * PJPS
        last_norms = pending_norms + drain_pend(gidx, limit=0,
                                                inline_norms=False)
        pending_norms = []
        for op in wo_queue:
            op()
        for eq, nrm in last_norms[:-2]:
            emit_norm(CT_t[eq], *nrm)
        for op in passa:
            op()
        CTl = CT_t[NQS - 1]
        if len(last_norms) >= 2 and last_norms[-2][1][0] == 2:
            emit_fast_tail(CTl, last_norms[-2][1], last_norms[-1][1])
        else:
            for eq, nrm in last_norms[-2:]:
                emit_norm(CT_t[eq], *nrm)
            for si in range(SI):
                sc = SI * (NQS - 1) + si
                for dh in range(D // SP):
                    i = si * (D // SP) + dh
                    po = psum.tile([128, SP], F32, tag="wo", name="po")
                    nc.tensor.matmul(
                        po[:], CTl[:, 1, si * 128:(si + 1) * 128],
                        wo_sb[:, 1, dh * SP:(dh + 1) * SP],
                        start=True, stop=True)
                    ob = obp.tile([128, SP], F32, name="ob")
                    nc.vector.tensor_tensor(ob[:], woa[i][:], po[:], ADD)
                    nc.sync.dma_start(
                        out_e.ap()[sc * 128:(sc + 1) * 128,
                                   dh * SP:(dh + 1) * SP], ob[:])

    nc.compile()
    return nc


_NC_CACHE = {}


def get_nc(S):
    if S not in _NC_CACHE:
        _NC_CACHE[S] = build(S)
    return _NC_CACHE[S]


def make_in_maps(x, W_q, b_q, W_k, b_k, W_v, b_v, W_o, b_o):
    B, S, _ = x.shape
    bf = ml_dtypes.bfloat16
    in_maps = []
    for core in range(NCORES):
        b, g = divmod(core, GPB)
        sl = slice(HD * g, HD * (g + 1))
        wv_aug = np.zeros((D, HPG * HAUG), np.float32)
        wv_block = np.asarray(W_v[:, sl]).reshape(D, HPG, DK)
        wv_aug.reshape(D, HPG, HAUG)[:, :, :DK] = wv_block
        def wtile_m(w):
            # [D, 256] -> [128, 2, D//128, 128] m-major chunk layout
            return np.ascontiguousarray(
                np.asarray(w).reshape(D // 128, 128, 2, 128)
                .transpose(1, 2, 0, 3))
        def wtile(w):
            # [D, N] -> [128, D//128, N] partition-major chunk layout
            return np.ascontiguousarray(
                np.asarray(w).reshape(D // 128, 128, -1).transpose(1, 0, 2))
        in_maps.append({
            "xT": np.ascontiguousarray(
                np.asarray(x[b]).T.reshape(D // 128, 128, S // SP, SP)
                .transpose(2, 1, 0, 3)).astype(bf),
            "wq": wtile_m(W_q[:, sl]).astype(bf),
            "wk": wtile_m(W_k[:, sl]).astype(bf),
            "wv": wtile(wv_aug).astype(bf),
            "wo": np.ascontiguousarray(
                np.asarray(W_o[sl, :]).reshape(2, 128, D)
                .transpose(1, 0, 2)).astype(bf),
            "bq": np.ascontiguousarray(
                np.asarray(b_q[sl]).reshape(2, 128).T).astype(np.float32),
            "bk": np.ascontiguousarray(
                np.asarray(b_k[sl]).reshape(2, 128).T).astype(np.float32),
        })
    return in_maps


def unshard(results, x, W_o, b_v, b_o):
    B, S, _ = x.shape
    out = np.zeros((B, S, D), np.float32)
    for core in range(NCORES):
        b = core // GPB
        out[b] += results[core]["out"]
    const = np.asarray(b_v).astype(np.float64) @ np.asarray(W_o).astype(np.float64)
    const += np.asarray(b_o).astype(np.float64)
    out += const.astype(np.float32)[None, None, :]
    return out


def run(inputs, trace=False):
    x = np.asarray(inputs["x"])
    nc = get_nc(x.shape[1])
    in_maps = make_in_maps(
        x, inputs["W_q"], inputs["b_q"], inputs["W_k"], inputs["b_k"],
        inputs["W_v"], inputs["b_v"], inputs["W_o"], inputs["b_o"])
    def attempt():
        res = run_bass_kernel_spmd(
            nc, in_maps, core_ids=list(range(NCORES)), trace=trace)
        # force materialization here: PJRT surfaces device errors lazily
        res.results = [{k: np.asarray(v) for k, v in r.items()}
                       for r in res.results]
        return res
    try:
        res = attempt()
    except Exception:
        # transient device errors (e.g. NRT_EXEC_UNIT_UNRECOVERABLE) clear
        # on re-execution of the same NEFF
        res = attempt()
    out = unshard(res.results, x, inputs["W_o"], inputs["b_v"], inputs["b_o"])
    return out, res


def kernel(**inputs):
    out, _ = run(inputs, trace=False)
    return out

